# revision 1
# baseline (speedup 1.0000x reference)
"""Trainium2 Bass kernel for nn_ConstrainedEnhancementModel.

Contract: kernel(**inputs) takes the FULL unsharded inputs (as produced by
reference.setup_inputs()) and returns the FULL [4096, 2000, 6] float32 output.

Strategy (pure data parallel over 8 NeuronCores, 512 batch rows each):
  - Feature-major MLP chain: every hidden activation is stored [feat, batch]
    so torch-layout weights [fan_in, fan_out] are directly the matmul lhsT.
  - The final layer flips to batch-major: lhsT = h5 (feature-major) slices,
    rhs = W6 tiles, so output DMA writes are contiguous.
  - The constraint/interpolation epilogue is folded into the final matmul:
        out = h5 @ (W6 * c_dec) + x @ G + ones * (b6 * c_dec)
    where G is a sparse constant [600, 12000] matrix holding the linear
    interpolation + anchor/blend coefficients.  G contributions are exact
    f32 (anchor timesteps reproduce the input bit-exactly); the decoded
    path is bf16 (it only ever enters scaled by 0.2 or in the tail).
"""

import numpy as np
import ml_dtypes

import bass_rust
import concourse.bass as bass
import concourse.bacc as bacc
import concourse.mybir as mybir
import concourse.tile as tile
from concourse import bass_utils

F32 = mybir.dt.float32
BF16 = mybir.dt.bfloat16
BF16_NP = ml_dtypes.bfloat16

# Problem config (hardcoded; must match the reference)
LOW_T = 100
HIGH_T = 2000
FEAT = 6
HID = 256
NUM_CLASSES = 10
LBL_DIM = 16
UP = 20
B = 4096
NCORES = 8
BC = B // NCORES          # 512 batch rows per core
NBT = BC // 128           # 4 batch tiles per core
D_IN = LOW_T * FEAT       # 600
D_OUT = HIGH_T * FEAT     # 12000
NW = 25                   # output windows (80 timesteps * 6 feats = 480 cols)
WT = 480
NI4 = 7                   # ceil(25/4) groups of 4 windows


def _build_nc():
    """Build the single-core Bass program (SPMD: same program on all 8)."""
    nc = bacc.Bacc("TRN2", target_bir_lowering=False, debug=False)

    x_d = nc.dram_tensor("x", [BC, 608], F32, kind="ExternalInput")
    lab_d = nc.dram_tensor("labf", [1, BC], BF16, kind="ExternalInput")
    w1_d = nc.dram_tensor("w1re", [NI4, 128, 512], BF16, kind="ExternalInput")
    w2_d = nc.dram_tensor("w2", [512, 256], BF16, kind="ExternalInput")
    w3_d = nc.dram_tensor("w3", [256, 128], BF16, kind="ExternalInput")
    w4a_d = nc.dram_tensor("w4a", [128, 256], BF16, kind="ExternalInput")
    w4b_d = nc.dram_tensor("w4b", [16, 256], BF16, kind="ExternalInput")
    w5_d = nc.dram_tensor("w5", [256, 512], BF16, kind="ExternalInput")
    w6_d = nc.dram_tensor("w6p", [512, D_OUT], BF16, kind="ExternalInput")
    b1_d = nc.dram_tensor("b1", [512, 1], F32, kind="ExternalInput")
    b2_d = nc.dram_tensor("b2", [256, 1], F32, kind="ExternalInput")
    b3_d = nc.dram_tensor("b3", [128, 1], F32, kind="ExternalInput")
    b4_d = nc.dram_tensor("b4", [256, 1], F32, kind="ExternalInput")
    b5_d = nc.dram_tensor("b5", [512, 1], F32, kind="ExternalInput")
    emb_d = nc.dram_tensor("embT", [NUM_CLASSES, LBL_DIM], BF16, kind="ExternalInput")
    iota_d = nc.dram_tensor("iota10", [NUM_CLASSES, 1], F32, kind="ExternalInput")
    id_d = nc.dram_tensor("ident", [128, 128], F32, kind="ExternalInput")
    g_d = nc.dram_tensor("gmat", [128, NI4 * WT], BF16, kind="ExternalInput")
    ones_d = nc.dram_tensor("onesrow", [2, NI4 * 512], BF16, kind="ExternalInput")
    y_d = nc.dram_tensor("y", [BC, D_OUT], F32, kind="ExternalOutput")

    RELU = mybir.ActivationFunctionType.Relu
    IDENT = mybir.ActivationFunctionType.Identity

    with tile.TileContext(nc) as tc:
        with (
            tc.tile_pool(name="const", bufs=1) as cp,
            tc.tile_pool(name="w6pool", bufs=3) as wp,
            tc.tile_pool(name="outpool", bufs=8) as op,
            tc.tile_pool(name="ppool", bufs=8, space="PSUM") as pm,
        ):
            # ---- persistent SBUF tensors ----
            cw1 = [cp.tile([128, 512], BF16, tag=f"cw1_{i}", name=f"cw1_{i}") for i in range(NI4)]
            cw2 = [cp.tile([128, 256], BF16, tag=f"cw2_{i}", name=f"cw2_{i}") for i in range(4)]
            cw3 = [cp.tile([128, 128], BF16, tag=f"cw3_{i}", name=f"cw3_{i}") for i in range(2)]
            cw4a = cp.tile([128, 256], BF16, tag="cw4a", name="cw4a")
            cw4b = cp.tile([16, 256], BF16, tag="cw4b", name="cw4b")
            cw5 = [cp.tile([128, 512], BF16, tag=f"cw5_{i}", name=f"cw5_{i}") for i in range(2)]
            cb1 = [cp.tile([128, 1], F32, tag=f"cb1_{i}", name=f"cb1_{i}") for i in range(4)]
            cb2 = [cp.tile([128, 1], F32, tag=f"cb2_{i}", name=f"cb2_{i}") for i in range(2)]
            cb3 = cp.tile([128, 1], F32, tag="cb3", name="cb3")
            cb4 = [cp.tile([128, 1], F32, tag=f"cb4_{i}", name=f"cb4_{i}") for i in range(2)]
            cb5 = [cp.tile([128, 1], F32, tag=f"cb5_{i}", name=f"cb5_{i}") for i in range(4)]
            cemb = cp.tile([NUM_CLASSES, LBL_DIM], BF16, tag="cemb", name="cemb")
            ciota = cp.tile([NUM_CLASSES, 1], F32, tag="ciota", name="ciota")
            cident = cp.tile([128, 128], F32, tag="cident", name="cident")
            cg = cp.tile([128, NI4 * WT], BF16, tag="cg", name="cg")
            clab = cp.tile([1, BC], BF16, tag="clab", name="clab")
            ones10 = cp.tile([1, NUM_CLASSES], BF16, tag="ones10", name="ones10")
            xre_b = cp.tile([128, NI4 * 512], BF16, tag="xre_b", name="xre_b")
            xsb = [cp.tile([128, 608], F32, tag=f"xsb_{i}", name=f"xsb_{i}") for i in range(NBT)]
            h1 = [cp.tile([128, BC], BF16, tag=f"h1_{i}", name=f"h1_{i}") for i in range(4)]
            h2 = [cp.tile([128, BC], BF16, tag=f"h2_{i}", name=f"h2_{i}") for i in range(2)]
            feat = cp.tile([128, BC], BF16, tag="feat", name="feat")
            h4 = [cp.tile([128, BC], BF16, tag=f"h4_{i}", name=f"h4_{i}") for i in range(2)]
            h5 = [cp.tile([128, BC], BF16, tag=f"h5_{i}", name=f"h5_{i}") for i in range(4)]
            onehot = cp.tile([NUM_CLASSES, BC], BF16, tag="onehot", name="onehot")
            embt = cp.tile([LBL_DIM, BC], BF16, tag="embt", name="embt")

            # ---- const loads ----
            # tiny PE-gating transfers first (the PE runs in order, so the
            # label matmul + transposes stall on these if they queue behind
            # the W6 prefetch flood), then x, then everything else
            nc.sync.dma_start(clab[:], lab_d[:])
            nc.sync.dma_start(ciota[:], iota_d[:])
            nc.sync.dma_start(cemb[:], emb_d[:])
            nc.sync.dma_start(cident[:], id_d[:])
            for bt in range(NBT):
                nc.sync.dma_start(xsb[bt][:], x_d[bt * 128:(bt + 1) * 128, :])
            for i in range(NI4):
                nc.sync.dma_start(cw1[i][:], w1_d[i])
            for k in range(4):
                nc.sync.dma_start(cw2[k][:], w2_d[k * 128:(k + 1) * 128, :])
            for k in range(2):
                nc.sync.dma_start(cw3[k][:], w3_d[k * 128:(k + 1) * 128, :])
            nc.sync.dma_start(cw4a[:], w4a_d[:])
            nc.sync.dma_start(cw4b[:], w4b_d[:])
            for k in range(2):
                nc.sync.dma_start(cw5[k][:], w5_d[k * 128:(k + 1) * 128, :])
            for m in range(4):
                nc.sync.dma_start(cb1[m][:], b1_d[m * 128:(m + 1) * 128, :])
                nc.sync.dma_start(cb5[m][:], b5_d[m * 128:(m + 1) * 128, :])
            for m in range(2):
                nc.sync.dma_start(cb2[m][:], b2_d[m * 128:(m + 1) * 128, :])
                nc.sync.dma_start(cb4[m][:], b4_d[m * 128:(m + 1) * 128, :])
            nc.sync.dma_start(cb3[:], b3_d[:])
            nc.sync.dma_start(cg[:], g_d[:])
            # bias rows for G: row 30 of every 32-row group = 1.0, row 31 = 0
            # (dependency-free; transpose copies only write rows 0..29)
            for w in range(4):
                nc.sync.dma_start(xre_b[32 * w + 30:32 * w + 32, :], ones_d[:])
            nc.gpsimd.memset(ones10[:], 1.0)

            # block i4=6 only has one window (w'=0); zero the rest of its
            # partitions once so the L1 matmul never reads uninitialized SBUF
            # (the matching w1re rows are zero).
            for p0 in (32, 64, 96):
                nc.gpsimd.memset(xre_b[p0:p0 + 32, 6 * 512:7 * 512], 0.0)

            # ---- label one-hot + embedding (feature-major [16, BC]) ----
            psl = pm.tile([128, 512], F32, tag="ps", name="ps")
            nc.tensor.matmul(psl[0:NUM_CLASSES, 0:BC], ones10[:], clab[:],
                             start=True, stop=True)
            nc.vector.tensor_scalar(
                onehot[:], psl[0:NUM_CLASSES, 0:BC], ciota[:], None,
                mybir.AluOpType.is_equal,
            )
            pse = pm.tile([128, 512], F32, tag="ps", name="ps")
            nc.tensor.matmul(pse[0:LBL_DIM, 0:BC], cemb[:], onehot[:],
                             start=True, stop=True)
            nc.vector.tensor_copy(embt[:], pse[0:LBL_DIM, 0:BC])

            # ---- load x, transpose into window-blocked layout ----
            # Window i = 4*i4 + w' needs x columns 24i..24i+30 on partitions
            # 32w'..; transpose-mode matmuls must output at psum partition 0,
            # so the four window transposes of a block land side by side in
            # one psum tile and partition-shifting copies (0 -> 32w', both
            # 32-aligned) place them.
            for bt in range(NBT):
                for i4 in range(NI4):
                    nwin = 4 if i4 < 6 else 1
                    ps = pm.tile([128, 512], F32, tag="ps", name="ps")
                    for w in range(nwin):
                        nc.tensor.transpose(
                            ps[0:32, 128 * w:128 * w + 128],
                            xsb[bt][:, 96 * i4 + 24 * w:96 * i4 + 24 * w + 32],
                            cident[:],
                        )
                    dst = slice(i4 * 512 + bt * 128, i4 * 512 + (bt + 1) * 128)
                    for w in range(nwin):
                        if w % 2 == 0:
                            nc.vector.tensor_copy(
                                xre_b[32 * w:32 * w + 30, dst], ps[0:30, 128 * w:128 * w + 128]
                            )
                        else:
                            nc.scalar.copy(
                                xre_b[32 * w:32 * w + 30, dst], ps[0:30, 128 * w:128 * w + 128]
                            )


            # ---- encoder / decoder MLP (feature-major, N = BC) ----
            # L1: [600->512] via window-blocked x / rearranged W1
            for m in range(4):
                ps = pm.tile([128, 512], F32, tag="ps", name="ps")
                for i4 in range(NI4):
                    nc.tensor.matmul(
                        ps[:, 0:BC], cw1[i4][:, m * 128:(m + 1) * 128],
                        xre_b[:, i4 * 512:(i4 + 1) * 512],
                        start=(i4 == 0), stop=(i4 == NI4 - 1),
                    )
                if m % 2 == 0:
                    nc.scalar.activation(h1[m][:], ps[:, 0:BC], RELU, bias=cb1[m][:])
                else:
                    nc.vector.tensor_scalar(h1[m][:], ps[:, 0:BC], cb1[m][:], 0.0, mybir.AluOpType.add, mybir.AluOpType.max)
            # L2: [512->256]
            for m in range(2):
                ps = pm.tile([128, 512], F32, tag="ps", name="ps")
                for k in range(4):
                    nc.tensor.matmul(
                        ps[:, 0:BC], cw2[k][:, m * 128:(m + 1) * 128], h1[k][:],
                        start=(k == 0), stop=(k == 3),
                    )
                if m % 2 == 0:
                    nc.scalar.activation(h2[m][:], ps[:, 0:BC], RELU, bias=cb2[m][:])
                else:
                    nc.vector.tensor_scalar(h2[m][:], ps[:, 0:BC], cb2[m][:], 0.0, mybir.AluOpType.add, mybir.AluOpType.max)
            # L3: [256->128], no relu
            ps = pm.tile([128, 512], F32, tag="ps", name="ps")
            for k in range(2):
                nc.tensor.matmul(ps[:, 0:BC], cw3[k][:], h2[k][:],
                                 start=(k == 0), stop=(k == 1))
            nc.vector.tensor_scalar(feat[:], ps[:, 0:BC], cb3[:], None, mybir.AluOpType.add)
            # L4: [144->256] = feat part + label-embedding part
            for m in range(2):
                ps = pm.tile([128, 512], F32, tag="ps", name="ps")
                nc.tensor.matmul(ps[:, 0:BC], cw4a[:, m * 128:(m + 1) * 128],
                                 feat[:], start=True, stop=False)
                nc.tensor.matmul(ps[:, 0:BC], cw4b[:, m * 128:(m + 1) * 128],
                                 embt[:], start=False, stop=True)
                if m % 2 == 0:
                    nc.scalar.activation(h4[m][:], ps[:, 0:BC], RELU, bias=cb4[m][:])
                else:
                    nc.vector.tensor_scalar(h4[m][:], ps[:, 0:BC], cb4[m][:], 0.0, mybir.AluOpType.add, mybir.AluOpType.max)
            # L5: [256->512]
            for m in range(4):
                ps = pm.tile([128, 512], F32, tag="ps", name="ps")
                for k in range(2):
                    nc.tensor.matmul(
                        ps[:, 0:BC], cw5[k][:, m * 128:(m + 1) * 128], h4[k][:],
                        start=(k == 0), stop=(k == 1),
                    )
                if m % 2 == 0:
                    nc.scalar.activation(h5[m][:], ps[:, 0:BC], RELU, bias=cb5[m][:])
                else:
                    nc.vector.tensor_scalar(h5[m][:], ps[:, 0:BC], cb5[m][:], 0.0, mybir.AluOpType.add, mybir.AluOpType.max)

            # ---- final layer + fused constraint epilogue ----
            # Windows processed in blocks of 4 (one i4 group).  Per batch
            # tile: 4x4 W6 matmuls into four psum tiles, then the four K=32
            # G matmuls back-to-back -- they sit on distinct PE row groups
            # and distinct psum banks, so they run concurrently.
            for i4 in range(NI4):
                nwin = 4 if i4 < 6 else 1
                w6t = {}
                for w in range(nwin):
                    i = 4 * i4 + w
                    for k in range(4):
                        t = wp.tile([128, WT], BF16, tag=f"w6k_{w}_{k}", name=f"w6k_{w}_{k}", bufs=5)
                        nc.sync.dma_start(
                            t[:], w6_d[k * 128:(k + 1) * 128, i * WT:(i + 1) * WT]
                        )
                        w6t[(w, k)] = t
                for bt in range(NBT):
                    pss = []
                    for w in range(nwin):
                        ps = pm.tile([128, 512], F32, tag="ps", name="ps")[:, 0:WT]
                        pss.append(ps)
                        for k in range(4):
                            nc.tensor.matmul(
                                ps[:], h5[k][:, bt * 128:(bt + 1) * 128], w6t[(w, k)][:],
                                start=(k == 0), stop=False,
                            )
                    for w in range(nwin):
                        p0 = 32 * w
                        nc.tensor.matmul(
                            pss[w][:],
                            xre_b[p0:p0 + 32, i4 * 512 + bt * 128:i4 * 512 + (bt + 1) * 128],
                            cg[p0:p0 + 32, i4 * WT:(i4 + 1) * WT],
                            start=False, stop=True, tile_position=(p0, 0),
                        )
                    for w in range(nwin):
                        i = 4 * i4 + w
                        ob = op.tile([128, WT], F32, tag="ob", name="ob")
                        if (i * NBT + bt) % 2 == 0:
                            nc.vector.tensor_copy(ob[:], pss[w][:])
                        else:
                            nc.scalar.copy(ob[:], pss[w][:])
                        # anchor timesteps must equal the f32 input exactly
                        obap = ob[:]
                        dst_anchor = bass_rust.AP(
                            tensor=obap.tensor, offset=obap.offset,
                            ap=[[obap.ap[0][0], 128], [120, 4], [1, 6]],
                        )
                        nc.vector.tensor_copy(dst_anchor, xsb[bt][:, 24 * i:24 * i + 24])
                        nc.sync.dma_start(
                            y_d[bt * 128:(bt + 1) * 128, i * WT:(i + 1) * WT], ob[:]
                        )

    nc.compile()
    return nc


def _host_prep(inputs):
    """Build per-core in_maps from the full inputs."""
    x_full = np.asarray(inputs["low_res_data"], np.float32).reshape(B, D_IN)
    labels = np.asarray(inputs["labels"]).astype(np.float32)
    W1 = np.asarray(inputs["W1"], np.float32)
    W6 = np.asarray(inputs["W6"], np.float32)
    b6 = np.asarray(inputs["b6"], np.float32)

    # per-timestep blend coefficients (match the reference formulas)
    t = np.arange(HIGH_T)
    seg = np.clip(t // UP, 0, LOW_T - 2)
    alpha = ((t - seg * UP) / UP).astype(np.float64)
    is_anchor = (t % UP) == 0
    interior = t < (LOW_T - 1) * UP
    blendf = np.where(is_anchor, 1.0, np.where(interior, 0.8, 0.0))
    c_d = np.where(is_anchor, 0.0, np.where(interior, 0.2, 1.0))
    c_start = blendf * (1.0 - alpha)
    c_end = blendf * alpha

    # G matrix, window-blocked: [128, NI4*480]; window i lives at partition
    # offset 32*(i%4), col block i//4.  Rows r=0..29 <-> x col 24*i + r,
    # row 30 = bias row (paired with the constant-1.0 row of xre_f).
    gmat = np.zeros((128, NI4 * WT), np.float64)
    for tt in range(HIGH_T):
        i, dt = divmod(tt, 80)
        i4, wpos = divmod(i, 4)
        p0 = 32 * wpos
        sl = seg[tt] - 4 * i
        for f in range(FEAT):
            col = i4 * WT + FEAT * dt + f
            gmat[p0 + FEAT * sl + f, col] += c_start[tt]
            gmat[p0 + FEAT * (sl + 1) + f, col] += c_end[tt]
            gmat[p0 + 30, col] = c_d[tt] * np.float64(b6[FEAT * tt + f])
    gmat = gmat.astype(np.float32).astype(BF16_NP)

    c_d_full = np.repeat(c_d, FEAT).astype(np.float32)
    w6p = (W6 * c_d_full[None, :]).astype(BF16_NP)

    # W1 rearranged to the window-blocked x layout (duplicated/ones/pad rows
    # get zero weights)
    w1re = np.zeros((NI4, 128, 512), np.float32)
    for c in range(D_IN):
        i, r = divmod(c, 24)
        i4, wpos = divmod(i, 4)
        w1re[i4, 32 * wpos + r, :] = W1[c, :]
    w1re = w1re.astype(BF16_NP)

    const_map = {
        "w1re": w1re,
        "w2": np.asarray(inputs["W2"], np.float32).astype(BF16_NP),
        "w3": np.asarray(inputs["W3"], np.float32).astype(BF16_NP),
        "w4a": np.asarray(inputs["W4"], np.float32)[:128].astype(BF16_NP),
        "w4b": np.asarray(inputs["W4"], np.float32)[128:144].astype(BF16_NP),
        "w5": np.asarray(inputs["W5"], np.float32).astype(BF16_NP),
        "w6p": w6p,
        "b1": np.asarray(inputs["b1"], np.float32).reshape(512, 1),
        "b2": np.asarray(inputs["b2"], np.float32).reshape(256, 1),
        "b3": np.asarray(inputs["b3"], np.float32).reshape(128, 1),
        "b4": np.asarray(inputs["b4"], np.float32).reshape(256, 1),
        "b5": np.asarray(inputs["b5"], np.float32).reshape(512, 1),
        "embT": np.asarray(inputs["emb"], np.float32).astype(BF16_NP),
        "iota10": np.arange(NUM_CLASSES, dtype=np.float32).reshape(NUM_CLASSES, 1),
        "ident": np.eye(128, dtype=np.float32),
        "gmat": gmat,
        "onesrow": np.concatenate([np.ones((1, NI4 * 512), BF16_NP), np.zeros((1, NI4 * 512), BF16_NP)]),
    }

    in_maps = []
    for c in range(NCORES):
        sl = slice(c * BC, (c + 1) * BC)
        xc = np.zeros((BC, 608), np.float32)
        xc[:, 0:D_IN] = x_full[sl]
        m = dict(const_map)
        m["x"] = xc
        m["labf"] = labels[sl].reshape(1, BC).astype(BF16_NP)
        in_maps.append(m)
    return in_maps


_NC_CACHE = None


def kernel(**inputs) -> np.ndarray:
    global _NC_CACHE
    if _NC_CACHE is None:
        _NC_CACHE = _build_nc()
    nc = _NC_CACHE
    in_maps = _host_prep(inputs)
    res = bass_utils.run_bass_kernel_spmd(nc, in_maps, core_ids=list(range(NCORES)))
    out = np.concatenate([res.results[c]["y"] for c in range(NCORES)], axis=0)
    return out.reshape(B, HIGH_T, FEAT)



# revision 4
# speedup vs baseline: 2.1015x; 2.1015x over previous
"""Trainium2 Bass kernel for nn_ConstrainedEnhancementModel.

Contract: kernel(**inputs) takes the FULL unsharded inputs (as produced by
reference.setup_inputs()) and returns the FULL [4096, 2000, 6] float32 output.

Strategy (pure data parallel over 8 NeuronCores, 512 batch rows each):
  - Feature-major MLP chain in fp8 (e4m3) with DoubleRow matmuls: weights are
    scaled x64 into fp8's normal range, activations apply scale=1/64 on the
    psum read so stored activations stay raw-scale fp8.
  - x arrives host-side pre-transposed into the window-blocked layout
    (partition 32w+r = x col 24*(4*i4+w)+r, free = i4*512 + batch), in both
    fp8 (for L1) and bf16 (for the interpolation matmul).
  - Final layer: out = h5 @ (W6 * c_d * 256) + x @ (G * 256), evaluated per
    output window (480 cols); fp8 DoubleRow pairs for the W6 part, a K=32
    bf16 matmul on a 32-row PE tile for the G (lin-interp + b6) part.  The
    psum->sbuf copy applies 1/256 and writes bf16; output DMAs one
    [128, nwin*480] chunk per (group, batch-tile).
  - Output tensor is bf16 (within the rel-err budget); host upcasts to f32.
"""

import numpy as np
import ml_dtypes

import bass_rust
import concourse.bass as bass
import concourse.bacc as bacc
import concourse.mybir as mybir
import concourse.tile as tile
from concourse import bass_utils

F32 = mybir.dt.float32
BF16 = mybir.dt.bfloat16
F8 = mybir.dt.float8e4
BF16_NP = ml_dtypes.bfloat16
F8_NP = ml_dtypes.float8_e4m3fn

# Problem config (hardcoded; must match the reference)
LOW_T = 100
HIGH_T = 2000
FEAT = 6
HID = 256
NUM_CLASSES = 10
LBL_DIM = 16
UP = 20
B = 4096
NCORES = 8
BC = B // NCORES          # 512 batch rows per core
NBT = BC // 128           # 4 batch tiles per core
D_IN = LOW_T * FEAT       # 600
D_OUT = HIGH_T * FEAT     # 12000
NW = 25                   # output windows (80 timesteps * 6 feats = 480 cols)
WT = 480
NI4 = 7                   # ceil(25/4) groups of 4 windows
EW = 64.0                 # encoder weight fp8 scale
SC = 256.0                # decoder/W6/G fp8+psum scale
DR = mybir.MatmulPerfMode.DoubleRow

# wenc blob column offsets (fp8, ktile-major within each layer)
OW1 = 0            # 7 ktiles x 512
OW2 = 3584         # 4 ktiles x 256
OW3 = 4608         # 2 ktiles x 128
OW4 = 4864         # 2 ktiles x 256
OW5 = 5376         # 2 ktiles x 512
WENC = 6400


def _ap3(t, col_off, stride2, n3):
    """3-dim AP over all 128 partitions of tile t: [128, 2, n3]."""
    a = t[:]
    return bass_rust.AP(
        tensor=a.tensor, offset=a.offset + col_off,
        ap=[[a.ap[0][0], 128], [stride2, 2], [1, n3]],
    )


def _build_nc():
    """Build the single-core Bass program (SPMD: same program on all 8)."""
    nc = bacc.Bacc("TRN2", target_bir_lowering=False, debug=False)

    lab_d = nc.dram_tensor("labf", [1, BC], BF16, kind="ExternalInput")
    iota_d = nc.dram_tensor("iota10", [NUM_CLASSES, 1], F32, kind="ExternalInput")
    emb_d = nc.dram_tensor("embT", [NUM_CLASSES, LBL_DIM], BF16, kind="ExternalInput")
    x8_d = nc.dram_tensor("x8", [128, NI4 * 512], F8, kind="ExternalInput")
    xb_d = nc.dram_tensor("xb", [128, NI4 * 512], BF16, kind="ExternalInput")
    wenc_d = nc.dram_tensor("wenc", [128, WENC], F8, kind="ExternalInput")
    bias_d = nc.dram_tensor("biasb", [128, 13], F32, kind="ExternalInput")
    g_d = nc.dram_tensor("gmat", [128, NI4 * WT], BF16, kind="ExternalInput")
    w6_d = nc.dram_tensor("w6p", [128, NW * 4 * WT], F8, kind="ExternalInput")
    y_d = nc.dram_tensor("y", [BC, D_OUT], BF16, kind="ExternalOutput")

    RELU = mybir.ActivationFunctionType.Relu
    IDENT = mybir.ActivationFunctionType.Identity

    with tile.TileContext(nc) as tc:
        with (
            tc.tile_pool(name="const", bufs=1) as cp,
            tc.tile_pool(name="outpool", bufs=4) as op,
            tc.tile_pool(name="ppool", bufs=8, space="PSUM") as pm,
        ):
            # ---- persistent SBUF tensors ----
            clab = cp.tile([1, BC], BF16, tag="clab", name="clab")
            ciota = cp.tile([NUM_CLASSES, 1], F32, tag="ciota", name="ciota")
            cemb = cp.tile([NUM_CLASSES, LBL_DIM], BF16, tag="cemb", name="cemb")
            ones10 = cp.tile([1, NUM_CLASSES], BF16, tag="ones10", name="ones10")
            x8 = cp.tile([128, NI4 * 512], F8, tag="x8", name="x8")
            xb = cp.tile([128, NI4 * 512], BF16, tag="xb", name="xb")
            wenc = cp.tile([128, WENC], F8, tag="wenc", name="wenc")
            cbias = cp.tile([128, 13], F32, tag="cbias", name="cbias")
            cg = cp.tile([128, NI4 * WT], BF16, tag="cg", name="cg")
            w6all = cp.tile([128, NW * 4 * WT], F8, tag="w6all", name="w6all")
            h1 = cp.tile([128, 4 * BC], F8, tag="h1", name="h1")
            h2 = cp.tile([128, 2 * BC], F8, tag="h2", name="h2")
            l4r = cp.tile([128, 2 * BC], F8, tag="l4r", name="l4r")
            h4 = cp.tile([128, 2 * BC], F8, tag="h4", name="h4")
            h5 = cp.tile([128, 4 * BC], F8, tag="h5", name="h5")

            # ---- const loads (tiny label-path first: PE runs in order) ----
            nc.sync.dma_start(clab[:], lab_d[:])
            nc.sync.dma_start(ciota[:], iota_d[:])
            nc.sync.dma_start(cemb[:], emb_d[:])
            nc.sync.dma_start(x8[:], x8_d[:])
            nc.sync.dma_start(wenc[:], wenc_d[:])
            nc.sync.dma_start(cbias[:], bias_d[:])
            nc.sync.dma_start(xb[:], xb_d[:])
            nc.sync.dma_start(cg[:], g_d[:])
            for g in range(NI4):
                nwin = 4 if g < 6 else 1
                o = g * 4 * WT * 4
                nc.sync.dma_start(
                    w6all[:, o:o + nwin * 4 * WT], w6_d[:, o:o + nwin * 4 * WT]
                )
            nc.gpsimd.memset(ones10[:], 1.0)
            # l4r ktile1: zero whole region; emb rows 0-15 overwritten later
            nc.gpsimd.memset(l4r[:, BC:2 * BC], 0.0)

            # bias column views
            cb1 = [cbias[:, m:m + 1] for m in range(4)]
            cb2 = [cbias[:, 4 + m:5 + m] for m in range(2)]
            cb3 = cbias[:, 6:7]
            cb4 = [cbias[:, 7 + m:8 + m] for m in range(2)]
            cb5 = [cbias[:, 9 + m:10 + m] for m in range(4)]

            # ---- label one-hot + embedding -> l4r ktile1 rows 0-15 ----
            psl = pm.tile([128, 512], F32, tag="ps", name="ps")
            nc.tensor.matmul(psl[0:NUM_CLASSES, 0:BC], ones10[:], clab[:],
                             start=True, stop=True)
            onehot = cp.tile([NUM_CLASSES, BC], BF16, tag="onehot", name="onehot")
            nc.vector.tensor_scalar(
                onehot[:], psl[0:NUM_CLASSES, 0:BC], ciota[:], None,
                mybir.AluOpType.is_equal,
            )
            pse = pm.tile([128, 512], F32, tag="ps", name="ps")
            nc.tensor.matmul(pse[0:LBL_DIM, 0:BC], cemb[:], onehot[:],
                             start=True, stop=True)
            nc.vector.tensor_copy(l4r[0:LBL_DIM, BC:2 * BC], pse[0:LBL_DIM, 0:BC])

            # ---- encoder / decoder MLP (feature-major, fp8 DoubleRow) ----
            # L1: [600->512] window-blocked x, 7 ktiles = 3 DR pairs + 1 plain
            for m in range(4):
                ps = pm.tile([128, 512], F32, tag="ps", name="ps")
                for p in range(3):
                    nc.tensor.matmul(
                        ps[:, 0:BC],
                        _ap3(wenc, OW1 + 2 * p * 512 + m * 128, 512, 128),
                        _ap3(x8, 2 * p * 512, 512, 512),
                        start=(p == 0), stop=False, perf_mode=DR,
                    )
                nc.tensor.matmul(
                    ps[:, 0:BC], wenc[:, OW1 + 6 * 512 + m * 128:OW1 + 6 * 512 + (m + 1) * 128],
                    x8[:, 6 * 512:7 * 512], start=False, stop=True,
                )
                nc.scalar.activation(h1[:, m * BC:(m + 1) * BC], ps[:, 0:BC],
                                     RELU, bias=cb1[m], scale=1.0 / EW)
            # L2: [512->256], 4 ktiles = 2 DR pairs
            for m in range(2):
                ps = pm.tile([128, 512], F32, tag="ps", name="ps")
                for p in range(2):
                    nc.tensor.matmul(
                        ps[:, 0:BC],
                        _ap3(wenc, OW2 + 2 * p * 256 + m * 128, 256, 128),
                        _ap3(h1, 2 * p * BC, BC, 512),
                        start=(p == 0), stop=(p == 1), perf_mode=DR,
                    )
                nc.scalar.activation(h2[:, m * BC:(m + 1) * BC], ps[:, 0:BC],
                                     RELU, bias=cb2[m], scale=1.0 / EW)
            # L3: [256->128] no relu -> l4r ktile0
            ps = pm.tile([128, 512], F32, tag="ps", name="ps")
            nc.tensor.matmul(
                ps[:, 0:BC], _ap3(wenc, OW3, 128, 128), _ap3(h2, 0, BC, 512),
                start=True, stop=True, perf_mode=DR,
            )
            nc.scalar.activation(l4r[:, 0:BC], ps[:, 0:BC], IDENT,
                                 bias=cb3, scale=1.0 / EW)
            # L4: [144->256] (feat ktile + padded label ktile)
            for m in range(2):
                ps = pm.tile([128, 512], F32, tag="ps", name="ps")
                nc.tensor.matmul(
                    ps[:, 0:BC], _ap3(wenc, OW4 + m * 128, 256, 128),
                    _ap3(l4r, 0, BC, 512),
                    start=True, stop=True, perf_mode=DR,
                )
                nc.scalar.activation(h4[:, m * BC:(m + 1) * BC], ps[:, 0:BC],
                                     RELU, bias=cb4[m], scale=1.0 / EW)
            # L5: [256->512]
            for m in range(4):
                ps = pm.tile([128, 512], F32, tag="ps", name="ps")
                nc.tensor.matmul(
                    ps[:, 0:BC], _ap3(wenc, OW5 + m * 128, 512, 128),
                    _ap3(h4, 0, BC, 512),
                    start=True, stop=True, perf_mode=DR,
                )
                nc.scalar.activation(h5[:, m * BC:(m + 1) * BC], ps[:, 0:BC],
                                     RELU, bias=cb5[m], scale=1.0 / EW)

            # ---- final layer + fused constraint epilogue ----
            for i4 in range(NI4):
                nwin = 4 if i4 < 6 else 1
                for bt in range(NBT):
                    pss = []
                    for w in range(nwin):
                        ps = pm.tile([128, 512], F32, tag="ps", name="ps")[:, 0:WT]
                        pss.append(ps)
                        for k2 in range(2):
                            nc.tensor.matmul(
                                ps[:],
                                _ap3(h5, 2 * k2 * BC + bt * 128, BC, 128),
                                _ap3(w6all, (i4 * 4 + w) * 4 * WT + k2 * 2 * WT, WT, WT),
                                start=(k2 == 0), stop=False, perf_mode=DR,
                            )
                    for w in range(nwin):
                        p0 = 32 * w
                        nc.tensor.matmul(
                            pss[w],
                            xb[p0:p0 + 32, i4 * 512 + bt * 128:i4 * 512 + bt * 128 + 128],
                            cg[p0:p0 + 32, i4 * WT:(i4 + 1) * WT],
                            start=False, stop=True, tile_position=(p0, 0),
                        )
                    ob = op.tile([128, nwin * WT], BF16, tag=f"ob{nwin}", name=f"ob{nwin}")
                    for w in range(nwin):
                        if w % 2 == 0:
                            nc.scalar.mul(ob[:, w * WT:(w + 1) * WT], pss[w], 1.0 / SC)
                        else:
                            nc.vector.tensor_scalar_mul(
                                ob[:, w * WT:(w + 1) * WT], pss[w], 1.0 / SC)
                    nc.sync.dma_start(
                        y_d[bt * 128:(bt + 1) * 128,
                            i4 * 4 * WT:i4 * 4 * WT + nwin * WT],
                        ob[:],
                    )

    nc.compile()
    return nc


def _host_prep(inputs):
    """Build per-core in_maps from the full inputs."""
    x_full = np.asarray(inputs["low_res_data"], np.float32).reshape(B, D_IN)
    labels = np.asarray(inputs["labels"]).astype(np.float32)
    W6 = np.asarray(inputs["W6"], np.float32)
    b6 = np.asarray(inputs["b6"], np.float32)

    # per-timestep blend coefficients (match the reference formulas)
    t = np.arange(HIGH_T)
    seg = np.clip(t // UP, 0, LOW_T - 2)
    alpha = ((t - seg * UP) / UP).astype(np.float64)
    is_anchor = (t % UP) == 0
    interior = t < (LOW_T - 1) * UP
    blendf = np.where(is_anchor, 1.0, np.where(interior, 0.8, 0.0))
    c_d = np.where(is_anchor, 0.0, np.where(interior, 0.2, 1.0))
    c_start = blendf * (1.0 - alpha) * SC
    c_end = blendf * alpha * SC

    # G matrix, window-blocked: [128, NI4*480]; window i at partition
    # offset 32*(i%4), col block i//4.  Rows r=0..29 <-> x col 24*i + r,
    # row 30 = bias row (pairs with the 1.0 row of the x layout).
    gmat = np.zeros((128, NI4 * WT), np.float64)
    for tt in range(HIGH_T):
        i, dt = divmod(tt, 80)
        i4, wpos = divmod(i, 4)
        p0 = 32 * wpos
        sl = seg[tt] - 4 * i
        for f in range(FEAT):
            col = i4 * WT + FEAT * dt + f
            gmat[p0 + FEAT * sl + f, col] += c_start[tt]
            gmat[p0 + FEAT * (sl + 1) + f, col] += c_end[tt]
            gmat[p0 + 30, col] = c_d[tt] * SC * np.float64(b6[FEAT * tt + f])
    gmat = gmat.astype(np.float32).astype(BF16_NP)

    # W6 blob: [128, 96*480] fp8; window i block at col (i4*4+w)*1920,
    # sub-blocks [k2][ko] of 480 cols = W6 ktile (2*k2+ko) for that window.
    c_d_full = np.repeat(c_d, FEAT).astype(np.float32)
    w6s = (W6 * (c_d_full * SC)[None, :]).astype(np.float32)
    w6r = w6s.reshape(4, 128, NW, WT)
    w6blob = np.zeros((128, NW * 4 * WT), np.float32)
    for i in range(NW):
        i4, w = divmod(i, 4)
        for kt in range(4):
            o = (i4 * 4 + w) * 4 * WT + kt * WT
            w6blob[:, o:o + WT] = w6r[kt, :, i, :]
    w6blob = w6blob.astype(F8_NP)

    # encoder weight blob (fp8, x64, ktile-major)
    wenc = np.zeros((128, WENC), np.float32)
    W1 = np.asarray(inputs["W1"], np.float32)
    for c in range(D_IN):
        i, r = divmod(c, 24)
        i4, wpos = divmod(i, 4)
        wenc[32 * wpos + r, OW1 + i4 * 512:OW1 + (i4 + 1) * 512] = W1[c, :] * EW
    W2 = np.asarray(inputs["W2"], np.float32) * EW
    for kt in range(4):
        wenc[:, OW2 + kt * 256:OW2 + (kt + 1) * 256] = W2[kt * 128:(kt + 1) * 128]
    W3 = np.asarray(inputs["W3"], np.float32) * EW
    for kt in range(2):
        wenc[:, OW3 + kt * 128:OW3 + (kt + 1) * 128] = W3[kt * 128:(kt + 1) * 128]
    W4 = np.asarray(inputs["W4"], np.float32) * EW
    wenc[:, OW4:OW4 + 256] = W4[:128]
    wenc[0:16, OW4 + 256:OW4 + 512] = W4[128:144]
    W5 = np.asarray(inputs["W5"], np.float32) * EW
    for kt in range(2):
        wenc[:, OW5 + kt * 512:OW5 + (kt + 1) * 512] = W5[kt * 128:(kt + 1) * 128]
    wenc = wenc.astype(F8_NP)

    # bias blob [128, 13] f32
    biasb = np.zeros((128, 13), np.float32)
    biasb[:, 0:4] = np.asarray(inputs["b1"], np.float32).reshape(4, 128).T
    biasb[:, 4:6] = np.asarray(inputs["b2"], np.float32).reshape(2, 128).T
    biasb[:, 6] = np.asarray(inputs["b3"], np.float32)
    biasb[:, 7:9] = np.asarray(inputs["b4"], np.float32).reshape(2, 128).T
    biasb[:, 9:13] = np.asarray(inputs["b5"], np.float32).reshape(4, 128).T

    const_map = {
        "wenc": wenc,
        "biasb": biasb,
        "w6p": w6blob,
        "embT": np.asarray(inputs["emb"], np.float32).astype(BF16_NP),
        "iota10": np.arange(NUM_CLASSES, dtype=np.float32).reshape(NUM_CLASSES, 1),
        "gmat": gmat,
    }

    in_maps = []
    for c in range(NCORES):
        sl = slice(c * BC, (c + 1) * BC)
        xc = x_full[sl]  # [BC, 600]
        xw = np.zeros((128, NI4 * 512), np.float32)
        for i in range(NW):
            i4, wpos = divmod(i, 4)
            p0 = 32 * wpos
            ncols = min(30, D_IN - 24 * i)
            xw[p0:p0 + ncols, i4 * 512:i4 * 512 + BC] = xc[:, 24 * i:24 * i + ncols].T
            xw[p0 + 30, i4 * 512:i4 * 512 + BC] = 1.0
        m = dict(const_map)
        m["x8"] = xw.astype(F8_NP)
        m["xb"] = xw.astype(BF16_NP)
        m["labf"] = labels[sl].reshape(1, BC).astype(BF16_NP)
        in_maps.append(m)
    return in_maps


_NC_CACHE = None


def kernel(**inputs) -> np.ndarray:
    global _NC_CACHE
    if _NC_CACHE is None:
        _NC_CACHE = _build_nc()
    nc = _NC_CACHE
    in_maps = _host_prep(inputs)
    res = bass_utils.run_bass_kernel_spmd(nc, in_maps, core_ids=list(range(NCORES)))
    out = np.concatenate([res.results[c]["y"] for c in range(NCORES)], axis=0)
    return out.astype(np.float32).reshape(B, HIGH_T, FEAT)
